# revision 1
# baseline (speedup 1.0000x reference)
"""Single-head causal attention (B=4, S=4096, d_e=512, d_k=d_v=64) on 8 TRN2 cores.

SPMD: one program on all 8 cores; per-core behavior driven purely by input data.
  - core c handles batch b=c//2; the two cores of a batch split the 8 q-tiles
    (512 queries each) load-balanced: parity 0 -> q-tiles {0,2,5,7}, parity 1 ->
    {1,3,4,6} (equal causal work: 18 kv-tile interactions each, padded to 20).
  - bf16 datapath: x/weights converted host-side to bf16; all PE operands bf16
    (fp32 PSUM accumulation), enabling fast weight loads and halving DMA.
  - x^T arrives tile-major split in two SBUF halves (tiles 0-3 / 4-7); q
    projections read their parity's q-tiles via PE-side dynamic column offsets
    (values_load from moff), so no duplicated q input DMA.
  - Attention in "scores^T" layout: st[s,q] = k @ (q/sqrt(dk))^T. The softmax
    denominator rides the AV matmul via an appended ones column on v. Causal /
    padding masks are multiplicative {0,1} bf16 tiles picked from an SBUF
    palette at data-driven dynamic offsets, applied post-exp on DVE with the
    masked pairs' AV matmuls deferred to the group tail.
  - Per-group output: raw numerator rows + denominator row ([65, 512] fp32)
    DMA'd out right after the group's PSUM releases; the host does the final
    divide + transpose during assembly.
"""
import numpy as np
import ml_dtypes
from contextlib import ExitStack

import concourse.bass as bass
import concourse.tile as tile
from concourse import bacc, mybir
from concourse.bass_utils import run_bass_kernel_spmd

f32 = mybir.dt.float32
bf16 = mybir.dt.bfloat16
i32 = mybir.dt.int32
AF = mybir.ActivationFunctionType
ET = mybir.EngineType

B, S, DE, DK, DV = 4, 4096, 512, 64, 64
QT = 512                 # queries per group
NT = S // QT             # 8 s/q tiles per batch
NG = 4                   # groups (q-tiles) per core
NCH = DE // 128          # 4 contraction chunks
TW = NCH * QT            # tile width in sbuf cols (2048)
TQ = [[0, 2, 5, 7], [1, 3, 4, 6]]   # parity -> group -> q_tile index

# palette column offsets (elements): [drop(512) | keep(512) | tri master(896)]
PAL_DROP = 0
PAL_KEEP = 512
PAL_TRI0 = 1024 + 384    # tri for block blk is PAL_TRI0 - 128*blk


def build(kiter: int = 1, unroll: int = 1, skip_pads: bool = False):
    # skip_pads=True wraps each group's maybe-padding pairs in a tc.Switch;
    # it is numerically correct but the dispatch+reconverge cost on HW
    # (~8us per Switch) far exceeds the ~2us of skipped work, so it's off.
    nc = bacc.Bacc("TRN2", target_bir_lowering=False, debug=False)

    xt_d = nc.dram_tensor("xt", [NT, 128, TW], bf16, kind="ExternalInput").ap()
    wkv_d = nc.dram_tensor("wkv", [128, NCH * 128], bf16, kind="ExternalInput").ap()
    wq_d = nc.dram_tensor("wq", [128, NCH * DK], bf16, kind="ExternalInput").ap()
    moff_d = nc.dram_tensor("moff", [1, 40], i32, kind="ExternalInput").ap()
    tri_d = nc.dram_tensor("tri", [128, 896], bf16, kind="ExternalInput").ap()
    ident_d = nc.dram_tensor("ident", [65, 65], bf16, kind="ExternalInput").ap()
    out_d = nc.dram_tensor("out", [NG, 65, QT], f32, kind="ExternalOutput").ap()

    with tile.TileContext(nc) as tc, ExitStack() as ctx:
        per = ctx.enter_context(tc.tile_pool(name="persist", bufs=1))
        # PSUM pools: pkvt 2x[128,512] + pqo 2x[65,512] + ps 2x[128,1024]
        # = 8 banks, all coexisting (no cross-phase overlap deps)
        pkv_pool = ctx.enter_context(tc.tile_pool(name="pkv", bufs=2, space="PSUM"))
        pq_pool = ctx.enter_context(tc.tile_pool(name="pq", bufs=2, space="PSUM"))
        ps_pool = ctx.enter_context(tc.tile_pool(name="ps", bufs=2, space="PSUM"))

        vts_pool = ctx.enter_context(tc.tile_pool(name="vts", bufs=3))
        exp_pool = ctx.enter_context(tc.tile_pool(name="exp", bufs=12))

        # x^T tile-major, split in two halves so the dynamic q reads
        # (conservative whole-tensor deps) only gate half the DMAs:
        # both parities have exactly 2 q-tiles in tiles 0-3 and 2 in 4-7.
        xtsA = per.tile([128, 4 * TW], bf16)
        xtsB = per.tile([128, 4 * TW], bf16)
        wkv = per.tile([128, NCH * 128], bf16)
        wq = per.tile([128, NCH * DK], bf16)
        ident = per.tile([65, 65], bf16)
        pal = per.tile([128, 1920], bf16)
        kT = per.tile([128, S], bf16)   # rows 0:64 and 64:128 both hold k^T
        vaug = per.tile([128, S // 128, 65], bf16)   # per 128-kv block: [128,65]
        qTg = per.tile([128, NG * QT], bf16)  # duplicated rows like kT
        oTall = per.tile([65, NG * QT], f32)
        mofft = per.tile([1, 40], i32)

        # loop-invariant constants: identity, tri palette, mask palette
        nc.scalar.dma_start(mofft[:], moff_d[:])
        nc.scalar.dma_start(ident[:], ident_d[:])
        nc.scalar.dma_start(pal[:, 1024:1920], tri_d[:])
        nc.gpsimd.memset(pal[:, 0:512], 0.0)
        nc.gpsimd.memset(pal[:, 512:1024], 1.0)
        # mask palette + q-slot offsets load once into engine registers
        mv = [nc.values_load(mofft[0:1, i:i + 1].to_broadcast((1, 1)),
                             engines=[ET.DVE],
                             min_val=0, max_val=1920 - 512,
                             skip_runtime_bounds_check=True)
              for i in range(32)]
        # q-tile base column offsets (within xtsA for g<2, xtsB for g>=2)
        qv = [nc.values_load(mofft[0:1, 32 + g:33 + g].to_broadcast((1, 1)),
                             engines=[ET.PE],
                             min_val=0, max_val=3 * TW,
                             skip_runtime_bounds_check=True)
              for g in range(NG)]
        # per-group skip flag: 1 when the group's last 2 pairs are pure
        # padding (q-tile == 2g) -> their scores/exp are skipped entirely
        skipv = []
        if skip_pads:
            skipv = [nc.values_load(mofft[0:1, 36 + g:37 + g].to_broadcast((1, 1)),
                                    engines=[ET.PE, ET.Activation, ET.DVE, ET.Pool],
                                    min_val=0, max_val=1,
                                    skip_runtime_bounds_check=True)
                     for g in range(NG)]

        def body():

            # DMA emission order == transfer priority (all on the SP queue;
            # ACT stays clear for the exp stream)
            nc.sync.dma_start(wkv[:], wkv_d[:])
            nc.sync.dma_start(wq[:], wq_d[:])
            nc.sync.dma_start(xtsA[:, bass.ts(0, TW)], xt_d[0])
            nc.sync.dma_start(xtsA[:, bass.ts(1, TW)], xt_d[1])
            nc.sync.dma_start(xtsA[:, bass.ts(2, TW)], xt_d[2])
            nc.sync.dma_start(xtsA[:, bass.ts(3, TW)], xt_d[3])
            nc.sync.dma_start(xtsB[:, bass.ts(0, TW)], xt_d[4])
            nc.sync.dma_start(xtsB[:, bass.ts(1, TW)], xt_d[5])
            nc.sync.dma_start(xtsB[:, bass.ts(2, TW)], xt_d[6])
            nc.sync.dma_start(xtsB[:, bass.ts(3, TW)], xt_d[7])

            # ---- projections ------------------------------------------------
            def q_proj(g):
                src = xtsA if g < 2 else xtsB
                pq_t = pq_pool.tile([65, QT], f32, tag="pqo")
                pq = pq_t[0:64, :]
                for c in range(NCH):
                    nc.tensor.matmul(pq[:], wq[:, bass.ts(c, DK)],
                                     src[:, bass.ds(qv[g] + c * QT, QT)],
                                     start=(c == 0), stop=(c == NCH - 1))
                nc.vector.tensor_copy(qTg[0:64, bass.ts(g, QT)], pq[:])
                nc.vector.tensor_copy(qTg[64:128, bass.ts(g, QT)], pq[:])

            # k^T and v_aug for one s-tile
            def kv_proj(t):
                src, tt = (xtsA, t) if t < 4 else (xtsB, t - 4)
                pkv = pkv_pool.tile([128, QT], f32, tag="pkvt")
                for c in range(NCH):
                    nc.tensor.matmul(pkv[:], wkv[:, bass.ts(c, 128)],
                                     src[:, tt * TW + c * QT: tt * TW + (c + 1) * QT],
                                     start=(c == 0), stop=(c == NCH - 1))
                nc.vector.tensor_copy(kT[0:64, bass.ts(t, QT)], pkv[0:64, :])
                nc.vector.tensor_copy(kT[64:128, bass.ts(t, QT)], pkv[0:64, :])
                vts = vts_pool.tile([65, QT], bf16, tag="vts")
                nc.vector.tensor_copy(vts[0:64, :], pkv[64:128, :])
                nc.vector.memset(vts[64:65, :], 1.0)
                pvt = pkv_pool.tile([128, 4, 66], bf16, tag="pkvt")
                for blk in range(4):
                    nc.tensor.transpose(pvt[:, blk, 0:65],
                                        vts[:, bass.ts(blk, 128)],
                                        ident[:])
                nc.vector.tensor_copy(vaug[:, t * 4:(t + 1) * 4, :], pvt[:, :, 0:65])

            q_proj(0)

            # ---- attention, group-major, kv-projections just-in-time --------
            q_proj(1)
            kv_proj(0)
            kv_proj(1)
            def score_mm(ps, pi, g):
                for half in range(2):
                    sb = 2 * pi + half
                    rows = slice(64 * half, 64 * half + 64)
                    nc.tensor.matmul(ps[:, bass.ts(half, QT)],
                                     kT[rows, bass.ts(sb, 128)],
                                     qTg[rows, bass.ts(g, QT)],
                                     start=True, stop=True,
                                     tile_position=(64 * half, 0))

            def mask_mul(em, pi, g):
                for half in range(2):
                    rel = 2 * pi + half - 8 * g
                    nc.vector.tensor_mul(em[:, bass.ts(half, QT)],
                                         em[:, bass.ts(half, QT)],
                                         pal[:, bass.ds(mv[g * 8 + rel], QT)])

            for g in range(NG):
                npairs = 4 * g + 4
                po = pq_pool.tile([65, QT], f32, tag="pqo")
                # masked pairs (last 4 in index space) run first; their AV
                # matmuls are deferred to the group tail so the mask-multiply
                # latency stays off the PE chain.
                unm = list(range(4 * g))
                msk = list(range(4 * g, 4 * g + 4))
                order = (unm[:-1] + msk + unm[-1:]) if unm else msk
                av_emitted = [0]
                n_av = 2 * npairs
                deferred = []

                def emit_av(pi, em, g=g, po=po, av_emitted=av_emitted, n_av=n_av):
                    for half in range(2):
                        sb = 2 * pi + half
                        nc.tensor.matmul(po[:], vaug[:, sb, :],
                                         em[:, bass.ts(half, QT)],
                                         start=(av_emitted[0] == 0),
                                         stop=(av_emitted[0] == n_av - 1))
                        av_emitted[0] += 1

                def flush(pend, g=g, emit_av=emit_av, deferred=deferred):
                    ps, pi = pend
                    em = exp_pool.tile([128, 2 * QT], bf16)
                    nc.scalar.activation(em[:], ps[:], AF.Exp)
                    if pi >= 4 * g:   # masked pair: mults split DVE/POOL
                        mask_mul(em, pi, g)
                        deferred.append((pi, em))
                    else:
                        emit_av(pi, em)

                pending = None
                for pos, pi in enumerate(order):
                    ps = ps_pool.tile([128, 2 * QT], f32)
                    score_mm(ps, pi, g)
                    if pending is not None:
                        flush(pending)
                    pending = (ps, pi)
                    # prefetch next group's projections behind the first pairs
                    if g < NG - 1:
                        if pos == 0:
                            kv_proj(2 * g + 2)
                        elif pos == 1:
                            kv_proj(2 * g + 3)
                        elif pos == 2 and g < 2:
                            q_proj(g + 2)
                flush(pending)
                for pi, em in deferred:
                    emit_av(pi, em)

                # release po; raw numerator+denominator go straight out, the
                # host divides during assembly
                nc.vector.tensor_copy(oTall[:, bass.ts(g, QT)], po[:])
                nc.sync.dma_start(out_d[g], oTall[:, bass.ts(g, QT)])

        if kiter == 1:
            body()
        elif kiter < 0:   # python-unrolled (for TimelineSim steady-state)
            for _ in range(-kiter):
                body()
        else:
            with tc.For_i(0, kiter, 1, staggered_reset=True):
                for _ in range(unroll):
                    body()

    nc.compile()
    return nc


def _tile_cols(a):
    """[512, n*512] (d_e, cols) -> [n, 128, 4*512] tile-major host layout."""
    de, w = a.shape
    n = w // QT
    # out[t, p, c*QT + s] = a[c*128 + p, t*QT + s]
    return np.ascontiguousarray(
        a.reshape(NCH, 128, n, QT).transpose(2, 1, 0, 3).reshape(n, 128, NCH * QT))


def make_inputs(x, Wq, Wk, Wv):
    """Per-core input maps. x:[B,S,DE] f32; W*: [DE,64] f32."""
    bf = ml_dtypes.bfloat16
    wkv = np.concatenate([Wk, Wv], axis=1).astype(np.float32)          # [512,128]
    wqs = (Wq / np.float32(np.sqrt(DK))).astype(np.float32)            # [512,64]
    # weights chunk-major: [128, c*width + j] = W[c*128 + p, j]
    wkv_h = np.ascontiguousarray(
        wkv.reshape(NCH, 128, 128).transpose(1, 0, 2).reshape(128, NCH * 128)
    ).astype(bf)
    wq_h = np.ascontiguousarray(
        wqs.reshape(NCH, 128, DK).transpose(1, 0, 2).reshape(128, NCH * DK)
    ).astype(bf)
    ident = np.eye(65, dtype=bf)
    tri = (np.arange(896)[None, :] >= np.arange(128)[:, None] + 384).astype(bf)
    in_maps = []
    for core in range(8):
        b, p = core // 2, core % 2
        xt = np.ascontiguousarray(x[b].T, dtype=np.float32)            # [512, 4096]
        moff = np.zeros((1, 40), dtype=np.int32)
        for g in range(NG):
            t = TQ[p][g]
            moff[0, 32 + g] = (t % 4) * TW   # q slot base col in xtsA/xtsB
            moff[0, 36 + g] = 1 if t == 2 * g else 0   # last-2-pairs are padding
            for rel in range(8):
                j = 2 * g + rel // 4
                blk = rel % 4
                if j < t:
                    moff[0, g * 8 + rel] = PAL_KEEP
                elif j == t:
                    moff[0, g * 8 + rel] = PAL_TRI0 - 128 * blk
                else:
                    moff[0, g * 8 + rel] = PAL_DROP
        in_maps.append(dict(xt=_tile_cols(xt).astype(bf),
                            wkv=wkv_h, wq=wq_h, moff=moff, tri=tri, ident=ident))
    return in_maps


def assemble(results):
    out = np.empty((B, S, DV), dtype=np.float32)
    for core in range(8):
        b, p = core // 2, core % 2
        o = results[core]["out"]                      # [NG, 65, QT] f32
        for g in range(NG):
            t = TQ[p][g]
            num = o[g][0:DV, :]                       # [dv, q]
            den = o[g][DV, :]                         # [q]
            out[b, t * QT:(t + 1) * QT, :] = (num / den).T
    return out


_cache = {}


def _get_nc(kiter=1, unroll=1):
    if (kiter, unroll) not in _cache:
        _cache[(kiter, unroll)] = build(kiter, unroll)
    return _cache[(kiter, unroll)]


def run(x, Wq, Wk, Wv, kiter=1):
    nc = _get_nc(kiter)
    in_maps = make_inputs(x, Wq, Wk, Wv)
    res = run_bass_kernel_spmd(nc, in_maps, list(range(8)))
    return assemble(res.results)


def kernel(x, Wq, Wk, Wv):
    x = np.asarray(x, dtype=np.float32)
    return run(x, np.asarray(Wq, np.float32), np.asarray(Wk, np.float32),
               np.asarray(Wv, np.float32))



# revision 10
# speedup vs baseline: 1.0758x; 1.0758x over previous
"""Single-head causal attention (B=4, S=4096, d_e=512, d_k=d_v=64) on 8 TRN2 cores.

Dual-program design: the 8 q-tiles of each batch are split between two cores
as {0,2,5,7} / {1,3,4,6} (equal causal area).  Instead of one SPMD program
padded to the worst-case envelope (20 kv-tile interactions), we build TWO
static programs (one per parity) and launch them concurrently on disjoint
4-device meshes.  Everything is compile-time static, so each core does its
EXACT causal work (18 kv-tile interactions = 36 kv-block-pair units):

  - scores^T layout: st[s,q] = k @ (q/sqrt(dk))^T, kv blocks of 128 packed
    two-at-a-time into the PE via tile_position row halves (dup'd kT rows).
  - keep blocks processed in chunks of 3 ([128,1536] PSUM = 3 banks), one
    exp per chunk on ACT; the two diagonal pairs of each q-tile are packed
    into ONE [128,1280] ps tile with range-restricted scores/exp/AV, so the
    above-diagonal waste is never exp'd and masks shrink to four static
    [128,128] upper-tri multiplies on DVE.
  - softmax denominator rides the AV matmul via an appended ones column on
    v (out rows 65); host divides during assembly.
  - q projections use column-duplicated weights so the row-dup'd q^T needs a
    single [128,512] PSUM->SBUF copy.
"""
import numpy as np
import ml_dtypes
from contextlib import ExitStack

import concourse.bass as bass
import concourse.tile as tile
from concourse import bacc, mybir

f32 = mybir.dt.float32
bf16 = mybir.dt.bfloat16
AF = mybir.ActivationFunctionType

B, S, DE, DK, DV = 4, 4096, 512, 64, 64
QT = 512                 # queries per group (q-tile)
NCH = DE // 128          # 4 contraction chunks
TW = NCH * QT            # x-tile width in sbuf cols (2048)
TQ = [[0, 2, 5, 7], [1, 3, 4, 6]]   # parity -> q-tiles (ascending)
NG = 4

bfdt = ml_dtypes.bfloat16


def build(parity: int, kiter: int = 1, unroll: int = 1):
    tiles = TQ[parity]
    nkv = tiles[-1] + 1          # kv tiles needed: 8 (p0) / 7 (p1)
    nc = bacc.Bacc("TRN2", target_bir_lowering=False, debug=False)

    xt_d = nc.dram_tensor("xt", [nkv, 128, TW], bf16, kind="ExternalInput").ap()
    wkv_d = nc.dram_tensor("wkv", [128, NCH * 128], bf16, kind="ExternalInput").ap()
    wqd_d = nc.dram_tensor("wqd", [128, NCH * 128], bf16, kind="ExternalInput").ap()
    tri_d = nc.dram_tensor("tri", [128, 128], bf16, kind="ExternalInput").ap()
    ident_d = nc.dram_tensor("ident", [64, 64], bf16, kind="ExternalInput").ap()
    out_d = nc.dram_tensor("out", [NG, 65, QT], f32, kind="ExternalOutput").ap()

    with tile.TileContext(nc) as tc, ExitStack() as ctx:
        per = ctx.enter_context(tc.tile_pool(name="persist", bufs=1))
        # PSUM: ps 2x[128,1536] (6 banks) + po [65,512] (1) + pj [128,512] (1)
        ps_pool = ctx.enter_context(tc.tile_pool(name="ps", bufs=2, space="PSUM"))
        po_pool = ctx.enter_context(tc.tile_pool(name="po", bufs=1, space="PSUM"))
        pj_pool = ctx.enter_context(tc.tile_pool(name="pj", bufs=1, space="PSUM"))
        em_pool = ctx.enter_context(tc.tile_pool(name="em", bufs=3))

        xts = per.tile([128, nkv * TW], bf16)
        wkv = per.tile([128, NCH * 128], bf16)
        wqd = per.tile([128, NCH * 128], bf16)
        tri = per.tile([128, 128], bf16)
        ident = per.tile([64, 64], bf16)
        kT = per.tile([128, nkv * QT], bf16)    # rows 0:64 / 64:128 dup'd k^T
        vts = per.tile([64, QT], bf16)
        vaug = per.tile([128, 4 * nkv, 65], bf16)  # per kv-block [128,65]; col 64 = ones
        qTg = per.tile([128, NG * QT], bf16)    # dup'd rows
        oTall = per.tile([65, NG * QT], f32)

        nc.scalar.dma_start(tri[:], tri_d[:])
        nc.scalar.dma_start(ident[:], ident_d[:])
        nc.gpsimd.memset(vaug[:, :, 64:65], 1.0)   # ones column, loop-invariant

        def body():
            # DMA emission order == priority, all on the SP HWDGE ring
            nc.sync.dma_start(wkv[:], wkv_d[:])
            nc.sync.dma_start(wqd[:], wqd_d[:])
            for s in range(nkv):
                nc.sync.dma_start(xts[:, bass.ts(s, TW)], xt_d[s])

            def q_proj(g):
                t = tiles[g]
                pq = pj_pool.tile([128, QT], f32, tag="pj")
                for c in range(NCH):
                    nc.tensor.matmul(pq[:], wqd[:, bass.ts(c, 128)],
                                     xts[:, t * TW + c * QT: t * TW + (c + 1) * QT],
                                     start=(c == 0), stop=(c == NCH - 1))
                nc.vector.tensor_copy(qTg[:, bass.ts(g, QT)], pq[:])

            def kv_proj(s):
                pkv = pj_pool.tile([128, QT], f32, tag="pj")
                for c in range(NCH):
                    nc.tensor.matmul(pkv[:], wkv[:, bass.ts(c, 128)],
                                     xts[:, s * TW + c * QT: s * TW + (c + 1) * QT],
                                     start=(c == 0), stop=(c == NCH - 1))
                # vts first: the transposes (next PE ops) wait only on it
                nc.vector.tensor_copy(vts[:], pkv[64:128, :])
                nc.vector.tensor_copy(kT[0:64, bass.ts(s, QT)], pkv[0:64, :])
                nc.vector.tensor_copy(kT[64:128, bass.ts(s, QT)], pkv[0:64, :])
                # pvt borrows a score-ring slot (PSUM budget: pj stays 1 bank)
                pvt = ps_pool.tile([128, 4, 64], bf16, tag="ps")
                for blk in range(4):
                    nc.tensor.transpose(pvt[:, blk, :],
                                        vts[:, bass.ts(blk, 128)], ident[:])
                nc.vector.tensor_copy(vaug[:, s * 4:(s + 1) * 4, 0:64], pvt[:])

            # ---- static unit/chunk schedule ---------------------------------
            # chunk = (kind, g, data):
            #   kind "keep": data = list of kv block indices (<=3)
            #   kind "diag": data = t (diag pairs packed into one [128,1280] ps)
            av_state = {}

            def rowsl(b):
                h = b % 2
                return slice(64 * h, 64 * h + 64)

            def score_keep(ps, g, blks):
                for i, b in enumerate(blks):
                    nc.tensor.matmul(ps[:, bass.ts(i, QT)],
                                     kT[rowsl(b), bass.ts(b, 128)],
                                     qTg[rowsl(b), bass.ts(g, QT)],
                                     start=True, stop=True,
                                     tile_position=(64 * (b % 2), 0))

            # diag segment table: per pair-half j: (ps offset, width, q offset,
            # mask?, po offset).  Blocks are 4t+j.  Offsets are packed so no
            # matmul output crosses a 512-f32 PSUM bank boundary while the
            # whole [0:1280] region stays contiguous for a single exp.
            DSEG = [(0, 512, 0, True, 0), (512, 384, 128, True, 128),
                    (1024, 256, 256, True, 256), (896, 128, 384, True, 384)]

            def score_diag(ps, g, t):
                for j, (po_, w, qo, _mk, _avo) in enumerate(DSEG):
                    b = 4 * t + j
                    nc.tensor.matmul(ps[:, po_:po_ + w],
                                     kT[rowsl(b), bass.ts(b, 128)],
                                     qTg[rowsl(b), g * QT + qo: (g + 1) * QT],
                                     start=True, stop=True,
                                     tile_position=(64 * (b % 2), 0))

            def emit_av(g, blks_or_t, em, kind):
                po = av_state[g]
                if kind == "keep":
                    for i, b in enumerate(blks_or_t):
                        nc.tensor.matmul(po[:, 0:QT], vaug[:, b, :],
                                         em[:, bass.ts(i, QT)],
                                         start=av_state[(g, "n")] == 0, stop=False)
                        av_state[(g, "n")] += 1
                else:
                    t = blks_or_t
                    for j, (po_, w, qo, _mk, avo) in enumerate(DSEG):
                        b = 4 * t + j
                        nc.tensor.matmul(po[:, avo:QT], vaug[:, b, :],
                                         em[:, po_:po_ + w],
                                         start=av_state[(g, "n")] == 0,
                                         stop=(j == 3))
                        av_state[(g, "n")] += 1

            def flush(pend):
                kind, g, data, ps = pend
                em = em_pool.tile([128, 3 * QT], bf16)
                if kind == "keep":
                    n = len(data)
                    nc.scalar.activation(em[:, 0:n * QT], ps[:, 0:n * QT], AF.Exp)
                    emit_av(g, data, em, "keep")
                else:
                    nc.scalar.activation(em[:, 0:1280], ps[:, 0:1280], AF.Exp)
                    for (po_, w, qo, mk, avo) in DSEG:
                        nc.vector.tensor_mul(em[:, po_:po_ + 128],
                                             em[:, po_:po_ + 128], tri[:])
                    emit_av(g, data, em, "diag")
                    # group epilogue: raw numerator+denominator out; host divides
                    po = av_state[g]
                    nc.vector.tensor_copy(oTall[:, bass.ts(g, QT)], po[:])
                    nc.sync.dma_start(out_d[g], oTall[:, bass.ts(g, QT)])

            # build chunk list + projection prefetch tasks
            chunks = []          # (kind, g, data)
            projs = []           # per-group: list of proj thunks needed BEFORE it
            prev_t = -1
            for g, t in enumerate(tiles):
                need = [lambda g=g: q_proj(g)]
                for s in range(prev_t + 1, t + 1):
                    need.append(lambda s=s: kv_proj(s))
                prev_t = t
                projs.append(need)
                nblk = 4 * t
                for c0 in range(0, nblk, 3):
                    chunks.append(("keep", g, list(range(c0, min(c0 + 3, nblk)))))
                chunks.append(("diag", g, t))

            # emission: software-pipelined; prefetch next group's projections
            # one per chunk while the current group streams.
            pending = None
            pending_projs = list(projs[0])
            group_of_chunk = [c[1] for c in chunks]
            for ci, (kind, g, data) in enumerate(chunks):
                # group entry: drain required projections, open po
                if g not in av_state:
                    for p in pending_projs:
                        p()
                    pending_projs = list(projs[g + 1]) if g + 1 < NG else []
                    po_t = po_pool.tile([65, QT], f32, tag="po")
                    av_state[g] = po_t
                    av_state[(g, "n")] = 0
                ps = ps_pool.tile([128, 3 * QT], f32, tag="ps")
                if kind == "keep":
                    score_keep(ps, g, data)
                else:
                    score_diag(ps, g, data)
                if pending is not None:
                    flush(pending)
                pending = (kind, g, data, ps)
                if pending_projs and (ci + 1 < len(chunks)
                                      and group_of_chunk[ci + 1] == g):
                    pending_projs.pop(0)()
            flush(pending)

        if kiter == 1:
            body()
        elif kiter < 0:   # python-unrolled (for TimelineSim steady-state)
            for _ in range(-kiter):
                body()
        else:
            with tc.For_i(0, kiter, 1, staggered_reset=True):
                for _ in range(unroll):
                    body()

    nc.compile()
    return nc


def _tile_cols(a, n):
    """[512, n*512] (d_e, cols) -> [n, 128, 2048] tile-major host layout."""
    de, w = a.shape
    return np.ascontiguousarray(
        a.reshape(NCH, 128, n, QT).transpose(2, 1, 0, 3).reshape(n, 128, NCH * QT))


def make_inputs(x, Wq, Wk, Wv, parity):
    """Per-batch input maps for one parity program. x:[B,S,DE] f32."""
    nkv = TQ[parity][-1] + 1
    wkv = np.concatenate([Wk, Wv], axis=1).astype(np.float32)          # [512,128]
    wqs = (Wq / np.float32(np.sqrt(DK))).astype(np.float32)            # [512,64]
    wkv_h = np.ascontiguousarray(
        wkv.reshape(NCH, 128, 128).transpose(1, 0, 2).reshape(128, NCH * 128)
    ).astype(bfdt)
    wqdup = np.concatenate([wqs, wqs], axis=1)                         # [512,128]
    wqd_h = np.ascontiguousarray(
        wqdup.reshape(NCH, 128, 128).transpose(1, 0, 2).reshape(128, NCH * 128)
    ).astype(bfdt)
    ident = np.eye(64, dtype=bfdt)
    tri = (np.arange(128)[None, :] >= np.arange(128)[:, None]).astype(bfdt)
    in_maps = []
    for b in range(B):
        xt = np.ascontiguousarray(x[b].T, dtype=np.float32)            # [512, 4096]
        in_maps.append(dict(xt=_tile_cols(xt, 8)[:nkv].astype(bfdt),
                            wkv=wkv_h, wqd=wqd_h, tri=tri, ident=ident))
    return in_maps


def assemble(res_by_parity):
    out = np.empty((B, S, DV), dtype=np.float32)
    for parity in range(2):
        for b in range(B):
            o = res_by_parity[parity][b]["out"]        # [NG, 65, QT]
            for g, t in enumerate(TQ[parity]):
                num = o[g][0:DV, :]
                den = o[g][DV, :]
                out[b, t * QT:(t + 1) * QT, :] = (num / den).T
    return out


# ---------------- PJRT execution (axon) ----------------------------------
_nc_cache = {}
_exec_cache = {}


def _get_nc(parity, kiter=1, unroll=1):
    key = (parity, kiter, unroll)
    if key not in _nc_cache:
        _nc_cache[key] = build(parity, kiter, unroll)
    return _nc_cache[key]


def _get_exec(parity, kiter=1, unroll=1):
    """Jitted 4-device shard_map executable for one parity program."""
    key = (parity, kiter, unroll)
    if key in _exec_cache:
        return _exec_cache[key]
    import jax
    from jax.sharding import Mesh, PartitionSpec
    from jax.experimental.shard_map import shard_map
    import concourse.bass2jax as b2j

    b2j.install_neuronx_cc_hook()
    nc = _get_nc(parity, kiter, unroll)
    partition_name = (nc.partition_id_tensor.name
                      if nc.partition_id_tensor else None)

    in_names, out_names, out_avals, zero_outs = [], [], [], []
    for alloc in nc.m.functions[0].allocations:
        if not isinstance(alloc, mybir.MemoryLocationSet):
            continue
        name = alloc.memorylocations[0].name
        if alloc.kind == "ExternalInput":
            if name != partition_name:
                in_names.append(name)
        elif alloc.kind == "ExternalOutput":
            shape = tuple(alloc.tensor_shape)
            dtype = mybir.dt.np(alloc.dtype)
            out_avals.append(jax.core.ShapedArray(shape, dtype))
            zero_outs.append(np.zeros(shape, dtype))
            out_names.append(name)
    n_params = len(in_names)
    n_outs = len(out_avals)
    all_in_names = list(in_names) + list(out_names)
    if partition_name is not None:
        all_in_names.append(partition_name)
    donate = tuple(range(n_params, n_params + n_outs))

    def _body(*args):
        operands = list(args)
        if partition_name is not None:
            operands.append(b2j.partition_id_tensor())
        outs = b2j._bass_exec_p.bind(
            *operands,
            out_avals=tuple(out_avals),
            in_names=tuple(all_in_names),
            out_names=tuple(out_names),
            lowering_input_output_aliases=(),
            sim_require_finite=True,
            sim_require_nnan=True,
            nc=nc,
        )
        return tuple(outs)

    devices = jax.devices()[4 * parity: 4 * parity + 4]
    mesh = Mesh(np.asarray(devices), ("core",))
    in_specs = (PartitionSpec("core"),) * (n_params + n_outs)
    out_specs = (PartitionSpec("core"),) * n_outs
    fn = jax.jit(
        shard_map(_body, mesh=mesh, in_specs=in_specs, out_specs=out_specs,
                  check_rep=False),
        donate_argnums=donate, keep_unused=True,
    )
    entry = (fn, mesh, list(in_names), out_names, zero_outs)
    _exec_cache[key] = entry
    return entry


def _dispatch(entry, in_maps):
    """Async-dispatch one parity program over its 4 devices."""
    import jax
    from jax.sharding import NamedSharding, PartitionSpec
    fn, mesh, in_names, out_names, zero_outs = entry
    sh = NamedSharding(mesh, PartitionSpec("core"))
    concat_in = [
        np.concatenate([np.asarray(in_maps[c][name]) for c in range(4)], axis=0)
        for name in in_names
    ]
    dev_in = [jax.device_put(a, sh) for a in concat_in]
    zo = [jax.device_put(np.zeros((4 * z.shape[0], *z.shape[1:]), z.dtype), sh)
          for z in zero_outs]
    out_arrs = fn(*dev_in, *zo)
    return out_arrs, out_names


def _collect(out_arrs, out_names, nshape):
    res = []
    for c in range(4):
        res.append({name: np.asarray(out_arrs[i]).reshape(4, *nshape[i])[c]
                    for i, name in enumerate(out_names)})
    return res


def run(x, Wq, Wk, Wv):
    entA = _get_exec(0)
    entB = _get_exec(1)
    imA = make_inputs(x, Wq, Wk, Wv, 0)
    imB = make_inputs(x, Wq, Wk, Wv, 1)
    arrsA, namesA = _dispatch(entA, imA)
    arrsB, namesB = _dispatch(entB, imB)
    shapes = [(NG, 65, QT)]
    resA = _collect(arrsA, namesA, shapes)
    resB = _collect(arrsB, namesB, shapes)
    return assemble([resA, resB])


def kernel(x, Wq, Wk, Wv):
    x = np.asarray(x, dtype=np.float32)
    return run(x, np.asarray(Wq, np.float32), np.asarray(Wk, np.float32),
               np.asarray(Wv, np.float32))
